# revision 51
# baseline (speedup 1.0000x reference)
"""Multi-head attention (B=2, S=2048, D=1024, H=16) on 8 TRN2 NeuronCores.

Sharding: batch (2) x head-groups (4 heads/core). Each core computes its
batch's QKV projections for its 4 heads, causal attention, and a partial
output projection over its head slice; the host sums the 4 partials per
batch and adds the output bias.

Layout strategy: everything runs in "transposed" orientation so no on-chip
transposes are needed:
  q2^T[dm, s] = Wq[dm,:] @ Q^T       (host supplies swizzled Q^T etc.)
  scores^T[j, si] = k^T.T @ q^T      (d_h contraction, 2 heads row-tiled)
  attn^T = exp(scores^T/8) * mask    (no row-max: |scores| < ~4)
  ctx^T+denom = [v | 1s].T @ attn^T  (64 ones columns replicate the softmax
                                      denominator across partitions 64:128,
                                      so normalize is a plain DVE recip+mul)
  out[s, n] = ctxn^T.T @ Wo^T
Matmuls in f16 (moving operand streams 2 cols/cycle). Emission interleaves
proj(sb+1) into attn(sb) so the PE never idles long enough for the HAM
clock-gate to re-throttle and the scalar engine's exp stream hides under PE
work. All DMA sources are host-swizzled to be contiguous per partition
(2-8KB lines) to keep descriptor counts low.
"""

import numpy as np

B, S, D, H, DH = 2, 2048, 1024, 16, 64
NCORES = 8
CORES_PER_BATCH = 4
HPC = H // CORES_PER_BATCH  # heads per core = 4
MMDT = "f16"  # "f32r" | "f16" | "bf16"
NSB = S // 512   # 4 si-blocks of 512
NST = S // 128   # 16 s-tiles / j-tiles of 128

TRACE = False  # test.py sets True to collect an NTFF profile
LAST_RESULT = None  # BassKernelResults from the last run (for test.py)

_built = {}


def _build(causal: bool, mmdt: str):
    key = (causal, mmdt)
    if key in _built:
        return _built[key]
    import concourse.mybir as mybir
    import concourse.tile as tile
    from concourse import bacc
    from concourse.bass import ts, ds

    f32 = mybir.dt.float32
    DT = {"f32r": mybir.dt.float32r, "f16": mybir.dt.float16,
          "bf16": mybir.dt.bfloat16}[mmdt]
    DTNP = {"f32r": f32, "f16": mybir.dt.float16, "bf16": mybir.dt.bfloat16}[mmdt]
    EXP = mybir.ActivationFunctionType.Exp

    nc = bacc.Bacc("TRN2")
    qt = nc.dram_tensor("qt", [NSB, 128, 8, 512], DTNP, kind="ExternalInput")
    kt = nc.dram_tensor("kt", [NSB, 128, 8, 512], DTNP, kind="ExternalInput")
    vt = nc.dram_tensor("vt", [NSB, 128, 8, 512], DTNP, kind="ExternalInput")
    wq = nc.dram_tensor("wq", [128, 8, 256], DTNP, kind="ExternalInput")
    wk = nc.dram_tensor("wk", [128, 8, 256], DTNP, kind="ExternalInput")
    wv = nc.dram_tensor("wv", [128, 8, 256], DTNP, kind="ExternalInput")
    wo = nc.dram_tensor("wo", [128, 2, D], DTNP, kind="ExternalInput")
    bq = nc.dram_tensor("bq", [128, 2], f32, kind="ExternalInput")
    bk = nc.dram_tensor("bk", [128, 2], f32, kind="ExternalInput")
    bv = nc.dram_tensor("bv", [1, HPC * DH], DTNP, kind="ExternalInput")
    if causal:
        mp = nc.dram_tensor("mp", [128, 2, 128], DTNP, kind="ExternalInput")
    else:
        mt = nc.dram_tensor("mt", [S, S], DTNP, kind="ExternalInput")
    out = nc.dram_tensor("out", [S, D], DTNP, kind="ExternalOutput")

    import contextlib
    with tile.TileContext(nc) as tc, contextlib.ExitStack() as ctx_pools:
        with (
            tc.tile_pool(name="persist", bufs=1) as pp,
            tc.tile_pool(name="sc_ps", bufs=2, space="PSUM") as sc_ps,
            tc.tile_pool(name="ctx_ps", bufs=1, space="PSUM") as ctx_ps,
            tc.tile_pool(name="mm_ps", bufs=2, space="PSUM") as mm_ps,
        ):
            # ---- weights on gpsimd (inputs own the sync queue) ----
            wp = ctx_pools.enter_context(tc.tile_pool(name="wproj", bufs=1))
            owp = ctx_pools.enter_context(tc.tile_pool(name="outw", bufs=1))
            wq_t = wp.tile([128, 8, 256], DT)
            nc.gpsimd.dma_start(out=wq_t, in_=wq[:, :, :].bitcast(DT))
            bq_t = pp.tile([128, 2], f32)
            nc.gpsimd.dma_start(out=bq_t, in_=bq[:, :])
            bk_t = pp.tile([128, 2], f32)
            nc.gpsimd.dma_start(out=bk_t, in_=bk[:, :])
            bv_t = pp.tile([1, HPC * DH], DT)
            nc.gpsimd.dma_start(out=bv_t, in_=bv[:, :].bitcast(DT))
            wk_t = wp.tile([128, 8, 256], DT)
            nc.gpsimd.dma_start(out=wk_t, in_=wk[:, :, :].bitcast(DT))
            wv_t = wp.tile([128, 8, 256], DT)
            nc.gpsimd.dma_start(out=wv_t, in_=wv[:, :, :].bitcast(DT))
            mp_t = None
            if causal:
                mp_t = pp.tile([128, 2, 128], DT, name="mp_t")
            wo_t = owp.tile([128, 2, D], DT)

            ones_c = pp.tile([1, 128], DT)
            nc.vector.memset(ones_c.bitcast(DTNP), 1.0)

            ap = ctx_pools.enter_context(tc.tile_pool(name="attn", bufs=5))
            smp = ctx_pools.enter_context(tc.tile_pool(name="small", bufs=3))
            mlp = None if causal else ctx_pools.enter_context(tc.tile_pool(name="mload", bufs=3))
            op = ctx_pools.enter_context(tc.tile_pool(name="outp", bufs=3))
            sp = ctx_pools.enter_context(tc.tile_pool(name="stream", bufs=1))

            q2t = [pp.tile([128, S], DT, tag=f"q2t{i}", name=f"q2t{i}") for i in range(2)]
            k2t = [pp.tile([128, S], DT, tag=f"k2t{i}", name=f"k2t{i}") for i in range(2)]
            # vaug[:, jt, h, 0:64] = v; [.., 64:128] = ones (denominator rows)
            vaug = pp.tile([128, NST, HPC, 128], DT)
            nc.vector.memset(vaug[:, :, :, ds(DH, DH)].bitcast(DTNP), 1.0)
            ctxt = [pp.tile([128, S], DT, tag=f"ctxt{i}", name=f"ctxt{i}") for i in range(2)]

            # ---- per-sb input DMA (double-buffered via bufs=2 tags) ----
            qkv_tiles = {}

            def emit_input_dma(sb):
                qs = sp.tile([128, 8, 512], DT, tag="qs", name="qs", bufs=2)
                ks = sp.tile([128, 8, 512], DT, tag="ks", name="ks", bufs=2)
                vs = sp.tile([128, 8, 512], DT, tag="vs", name="vs", bufs=2)
                nc.sync.dma_start(out=qs, in_=qt[sb].bitcast(DT))
                nc.sync.dma_start(out=ks, in_=kt[sb].bitcast(DT))
                (nc.sync if sb == 1 else nc.gpsimd).dma_start(out=vs, in_=vt[sb].bitcast(DT))
                qkv_tiles[sb] = (qs, ks, vs)

            def proj_unit_qk(sb, hp, which):
                qs, ks, vs = qkv_tiles[sb]
                if which == "q":
                    src, w_t, b_t, dst = qs, wq_t, bq_t, q2t
                else:
                    src, w_t, b_t, dst = ks, wk_t, bk_t, k2t
                ps = mm_ps.tile([128, 512], f32, tag="mm", name="ps")
                for c in range(8):
                    nc.tensor.matmul(
                        ps, w_t[:, c, ts(hp, 128)], src[:, c, :],
                        start=(c == 0), stop=(c == 7),
                    )
                nc.vector.tensor_scalar_add(
                    dst[hp][:, ts(sb, 512)], ps, b_t[:, ds(hp, 1)])

            def proj_unit_v(sb, st4):
                qs, ks, vs = qkv_tiles[sb]
                st = sb * 4 + st4
                psv = mm_ps.tile([128, 256], f32, tag="mm", name="psv")
                for c in range(8):
                    nc.tensor.matmul(
                        psv, vs[:, c, ts(st4, 128)], wv_t[:, c, :],
                        start=(c == 0), stop=False,
                    )
                nc.tensor.matmul(psv, ones_c, bv_t, start=False, stop=True)
                nc.vector.tensor_copy(
                    vaug[:, st, :, 0:DH],
                    psv.rearrange("p (h x) -> p h x", h=HPC),
                )

            def emit_proj(sb):
                for hp in range(2):
                    proj_unit_qk(sb, hp, "q")
                    proj_unit_qk(sb, hp, "k")
                for st4 in range(4):
                    proj_unit_v(sb, st4)
                qkv_tiles.pop(sb)

            fillq = []

            def fill_one():
                if fillq:
                    fillq.pop(0)()

            # ---- attention + deferred output projection ----
            outq = []  # deferred out-proj st units from sb-1

            def emit_out_unit(final=False):
                if not outq:
                    return
                st = outq.pop(0)
                ot = op.tile([128, 1024], DT, tag="ot", name="ot")
                for nb in range(2):
                    po = mm_ps.tile([128, 512], f32, tag="mm", name="po")
                    for k in range(2):
                        nc.tensor.matmul(
                            po, ctxt[k][:, ts(st, 128)],
                            wo_t[:, k, ts(nb, 512)],
                            start=(k == 0), stop=(k == 1),
                        )
                    # scalar is idle after the last exp; share the tail copies
                    if final and nb == 0:
                        nc.scalar.copy(ot[:, ts(nb, 512)], po)
                    else:
                        nc.vector.tensor_copy(ot[:, ts(nb, 512)], po)
                (nc.gpsimd if st % 2 else nc.sync).dma_start(
                    out=out[ts(st, 128), :].bitcast(DT), in_=ot)

            def emit_attn(sb):
                for hp in range(2):
                    jts = list(range(4 * sb + 4)) if causal else list(range(NST))
                    cps = [ctx_ps.tile([128, 512], f32, tag=f"ctx{a}", name=f"cps{a}")
                           for a in range(2)]
                    pending = None  # (jt, at) awaiting attn@v

                    def lo_of(j):
                        # columns si<j0 of tile (j, sb) are fully masked
                        if not causal:
                            return 0
                        return max(0, (j - 4 * sb) * 128)

                    def emit_attnv(pjt, pat, last):
                        lo = lo_of(pjt)
                        for a in range(2):
                            h = 2 * hp + a
                            nc.tensor.matmul(
                                cps[a][:, ds(lo, 512 - lo)],
                                vaug[:, pjt, h, :],
                                pat[:, ds(a * 512 + lo, 512 - lo)],
                                start=(pjt == jts[0]), stop=last,
                            )

                    for jt in jts:
                        sc = sc_ps.tile([128, 1024], f32, tag="sc")
                        straddle = causal and jt >= 4 * sb
                        lo = lo_of(jt)
                        mt_t = None
                        if not causal:
                            mt_t = mlp.tile([128, 512], DT, tag="mt")
                            nc.sync.dma_start(
                                out=mt_t,
                                in_=mt[ts(jt, 128), ts(sb, 512)].bitcast(DT))
                        for a in range(2):
                            nc.tensor.matmul(
                                sc[:, ds(a * 512 + lo, 512 - lo)],
                                k2t[hp][ds(a * 64, 64), ts(jt, 128)],
                                q2t[hp][ds(a * 64, 64), ds(sb * 512 + lo, 512 - lo)],
                                start=True, stop=True,
                                tile_position=(a * 64, 0),
                            )
                        # additive mask on the scores BEFORE exp (masked ->
                        # -60000 -> exp gives exact 0) so attn@v depends only
                        # on exp, keeping the in-order PE stream stall-free
                        if straddle:
                            # only the 128-col diagonal block is partially
                            # masked, and it is the same lower-triangle
                            # pattern for every straddle tile
                            nc.vector.tensor_add(
                                sc.rearrange("p (a x) -> p a x", a=2)[:, :, ds(lo, 128)],
                                sc.rearrange("p (a x) -> p a x", a=2)[:, :, ds(lo, 128)],
                                mp_t)
                        elif not causal:
                            for a in range(2):
                                nc.vector.tensor_add(
                                    sc[:, ts(a, 512)], sc[:, ts(a, 512)], mt_t)
                        at = ap.tile([128, 1024], DT, tag="at")
                        if lo == 0:
                            nc.scalar.activation(at, sc, EXP, scale=0.125)
                        else:
                            # one ACTIVATE over both head-halves via a 3D AP
                            nc.scalar.activation(
                                at.rearrange("p (a x) -> p a x", a=2)[:, :, ds(lo, 512 - lo)],
                                sc.rearrange("p (a x) -> p a x", a=2)[:, :, ds(lo, 512 - lo)],
                                EXP, scale=0.125)
                        if pending is not None:
                            emit_attnv(pending[0], pending[1], False)
                        pending = (jt, at)
                        emit_out_unit()
                    emit_attnv(pending[0], pending[1], True)
                    for a in range(2):
                        # rows 64:128 of cps all hold the softmax denominator
                        # (reciprocal_approx_fast cannot read PSUM on HW --
                        # stage through SBUF first)
                        dn = smp.tile([DH, 512], f32, tag="dn", name="dn")
                        nc.vector.tensor_copy(dn, cps[a][ds(DH, DH), :])
                        rd = smp.tile([DH, 512], f32, tag="rd", name="rd")
                        nc.vector.reciprocal_approx_fast(rd, dn)
                        nc.vector.tensor_mul(
                            ctxt[hp][ds(a * DH, DH), ts(sb, 512)],
                            cps[a][0:DH, :], rd)
                # queue this sb's out-proj units; drained during next sb
                outq.extend(range(4 * sb, 4 * sb + 4))

            # ---- interleaved emission: proj(sb+1) fills PE gaps of attn(sb) ----
            emit_input_dma(0)
            if causal:
                nc.gpsimd.dma_start(out=mp_t, in_=mp[:, :, :].bitcast(DT))
            nc.gpsimd.dma_start(out=wo_t, in_=wo[:, :, :].bitcast(DT))
            emit_input_dma(1)
            emit_proj(0)
            emit_input_dma(2)
            emit_attn(0)
            emit_proj(1)
            emit_input_dma(3)
            emit_attn(1)
            emit_proj(2)
            emit_attn(2)
            emit_proj(3)
            emit_attn(3)
            while outq:
                emit_out_unit(final=True)

            ctx_pools.close()

    nc.finalize()
    _built[key] = nc
    return nc


def _is_causal(masked: np.ndarray) -> bool:
    c = ~np.tril(np.ones((S, S), dtype=bool))
    return all(np.array_equal(masked[b], c) for b in range(masked.shape[0]))


def _swizzle_in(xT: np.ndarray) -> np.ndarray:
    # [D, S] -> [sb, p, c, s] with D = c*128+p, S = sb*512+s
    return np.ascontiguousarray(
        xT.reshape(8, 128, NSB, 512).transpose(2, 1, 0, 3))


def kernel(Q, K, V, masked, WQ_w, WQ_b, WK_w, WK_b, WV_w, WV_b, Wo_w, Wo_b):
    global LAST_RESULT
    from concourse.bass_utils import run_bass_kernel_spmd

    Q = np.asarray(Q, dtype=np.float32)
    K = np.asarray(K, dtype=np.float32)
    V = np.asarray(V, dtype=np.float32)
    masked = np.asarray(masked)
    causal = _is_causal(masked)
    nc = _build(causal, MMDT)
    if MMDT == "f16":
        npdt = np.float16
    elif MMDT == "bf16":
        import ml_dtypes
        npdt = ml_dtypes.bfloat16
    else:
        npdt = np.float32

    qT = [_swizzle_in(Q[b].T.astype(npdt)) for b in range(B)]
    kT = [_swizzle_in(K[b].T.astype(npdt)) for b in range(B)]
    vT = [_swizzle_in(V[b].T.astype(npdt)) for b in range(B)]

    if causal:
        # additive diagonal-block mask [p, a, x]: -60000 where p > x
        # (same lower-triangle pattern for both head-halves a)
        p = np.arange(128)[:, None, None]
        x = np.arange(128)[None, None, :]
        mp_full = np.ascontiguousarray(np.broadcast_to(
            np.where(p > x, np.float32(-60000.0), np.float32(0.0)),
            (128, 2, 128)).astype(npdt))
    else:
        mtb = [np.ascontiguousarray(
            np.where(masked[b].T, np.float32(-60000.0), np.float32(0.0)).astype(npdt))
            for b in range(B)]

    in_maps = []
    for c in range(NCORES):
        b = c // CORES_PER_BATCH
        h0 = (c % CORES_PER_BATCH) * HPC
        sel = slice(h0 * DH, (h0 + HPC) * DH)
        wo_pad = np.asarray(Wo_w).T[sel].reshape(2, 128, D)
        m = {
            "qt": qT[b], "kt": kT[b], "vt": vT[b],
            "wq": np.ascontiguousarray(
                np.asarray(WQ_w)[sel].T.astype(npdt).reshape(8, 128, 256).transpose(1, 0, 2)),
            "wk": np.ascontiguousarray(
                np.asarray(WK_w)[sel].T.astype(npdt).reshape(8, 128, 256).transpose(1, 0, 2)),
            "wv": np.ascontiguousarray(
                np.asarray(WV_w)[sel].T.astype(npdt).reshape(8, 128, 256).transpose(1, 0, 2)),
            "wo": np.ascontiguousarray(wo_pad.astype(npdt).transpose(1, 0, 2)),
            "bq": np.ascontiguousarray(np.asarray(WQ_b)[sel].reshape(2, 128).T.astype(np.float32)),
            "bk": np.ascontiguousarray(np.asarray(WK_b)[sel].reshape(2, 128).T.astype(np.float32)),
            "bv": np.ascontiguousarray(np.asarray(WV_b)[sel].reshape(1, HPC * DH).astype(npdt)),
        }
        if causal:
            m["mp"] = mp_full
        else:
            m["mt"] = mtb[b]
        in_maps.append(m)

    res = run_bass_kernel_spmd(nc, in_maps, core_ids=list(range(NCORES)), trace=TRACE)
    LAST_RESULT = res

    acc = np.zeros((B, S, D), np.float64)
    for c in range(NCORES):
        acc[c // CORES_PER_BATCH] += res.results[c]["out"].astype(np.float64)
    acc += np.asarray(Wo_b, dtype=np.float64)[None, None, :]
    return acc.astype(np.float32)


# revision 57
# speedup vs baseline: 1.2371x; 1.2371x over previous
"""Multi-head attention (B=2, S=2048, D=1024, H=16) on 8 TRN2 NeuronCores.

Sharding: batch (2) x head-groups (4 heads/core). Each core computes its
batch's QKV projections for its 4 heads, causal attention, and a partial
output projection over its head slice; the host sums the 4 partials per
batch and adds the output bias.

Layout strategy: everything runs in "transposed" orientation so no on-chip
transposes are needed:
  q2^T[dm, s] = Wq[dm,:] @ Q^T       (host supplies swizzled Q^T etc.)
  scores^T[j, si] = k^T.T @ q^T      (d_h contraction, 2 heads row-tiled)
  attn^T = exp(scores^T/8) * mask    (no row-max: |scores| < ~4)
  ctx^T+denom = [v | 1s].T @ attn^T  (64 ones columns replicate the softmax
                                      denominator across partitions 64:128,
                                      so normalize is a plain DVE recip+mul)
  out[s, n] = ctxn^T.T @ Wo^T
Matmuls in f16 (moving operand streams 2 cols/cycle). Emission interleaves
proj(sb+1) into attn(sb) so the PE never idles long enough for the HAM
clock-gate to re-throttle and the scalar engine's exp stream hides under PE
work. All DMA sources are host-swizzled to be contiguous per partition
(2-8KB lines) to keep descriptor counts low.
"""

import numpy as np

B, S, D, H, DH = 2, 2048, 1024, 16, 64
NCORES = 8
CORES_PER_BATCH = 4
HPC = H // CORES_PER_BATCH  # heads per core = 4
MMDT = "f16"  # "f32r" | "f16" | "bf16"
NSB = S // 512   # 4 si-blocks of 512
NST = S // 128   # 16 s-tiles / j-tiles of 128

TRACE = False  # test.py sets True to collect an NTFF profile
LAST_RESULT = None  # BassKernelResults from the last run (for test.py)

_built = {}


def _build(causal: bool, mmdt: str):
    key = (causal, mmdt)
    if key in _built:
        return _built[key]
    import concourse.mybir as mybir
    import concourse.tile as tile
    from concourse import bacc
    from concourse.bass import ts, ds

    f32 = mybir.dt.float32
    DT = {"f32r": mybir.dt.float32r, "f16": mybir.dt.float16,
          "bf16": mybir.dt.bfloat16}[mmdt]
    DTNP = {"f32r": f32, "f16": mybir.dt.float16, "bf16": mybir.dt.bfloat16}[mmdt]
    EXP = mybir.ActivationFunctionType.Exp

    nc = bacc.Bacc("TRN2")
    qt = nc.dram_tensor("qt", [NSB, 128, 8, 512], DTNP, kind="ExternalInput")
    kt = nc.dram_tensor("kt", [NSB, 128, 8, 512], DTNP, kind="ExternalInput")
    vt = nc.dram_tensor("vt", [NSB, 128, 8, 512], DTNP, kind="ExternalInput")
    wq = nc.dram_tensor("wq", [128, 8, 256], DTNP, kind="ExternalInput")
    wk = nc.dram_tensor("wk", [128, 8, 256], DTNP, kind="ExternalInput")
    wv = nc.dram_tensor("wv", [128, 8, 256], DTNP, kind="ExternalInput")
    wo = nc.dram_tensor("wo", [128, 2, D], DTNP, kind="ExternalInput")
    bq = nc.dram_tensor("bq", [128, 2], f32, kind="ExternalInput")
    bk = nc.dram_tensor("bk", [128, 2], f32, kind="ExternalInput")
    bv = nc.dram_tensor("bv", [1, HPC * DH], DTNP, kind="ExternalInput")
    if causal:
        mp = nc.dram_tensor("mp", [128, 4, 512], DTNP, kind="ExternalInput")
    else:
        mt = nc.dram_tensor("mt", [S, S], DTNP, kind="ExternalInput")
    out = nc.dram_tensor("out", [S, D], DTNP, kind="ExternalOutput")

    import contextlib
    with tile.TileContext(nc) as tc, contextlib.ExitStack() as ctx_pools:
        with (
            tc.tile_pool(name="persist", bufs=1) as pp,
            tc.tile_pool(name="sc_ps", bufs=2, space="PSUM") as sc_ps,
            tc.tile_pool(name="ctx_ps", bufs=1, space="PSUM") as ctx_ps,
            tc.tile_pool(name="mm_ps", bufs=2, space="PSUM") as mm_ps,
        ):
            # ---- weights on gpsimd (inputs own the sync queue) ----
            wp = ctx_pools.enter_context(tc.tile_pool(name="wproj", bufs=1))
            owp = ctx_pools.enter_context(tc.tile_pool(name="outw", bufs=1))
            wq_t = wp.tile([128, 8, 256], DT)
            nc.gpsimd.dma_start(out=wq_t, in_=wq[:, :, :].bitcast(DT))
            wk_t = wp.tile([128, 8, 256], DT)
            nc.gpsimd.dma_start(out=wk_t, in_=wk[:, :, :].bitcast(DT))
            wv_t = wp.tile([128, 8, 256], DT)
            nc.gpsimd.dma_start(out=wv_t, in_=wv[:, :, :].bitcast(DT))
            bq_t = pp.tile([128, 2], f32)
            nc.gpsimd.dma_start(out=bq_t, in_=bq[:, :])
            bk_t = pp.tile([128, 2], f32)
            nc.gpsimd.dma_start(out=bk_t, in_=bk[:, :])
            bv_t = pp.tile([1, HPC * DH], DT)
            nc.gpsimd.dma_start(out=bv_t, in_=bv[:, :].bitcast(DT))
            mp_t = None
            if causal:
                mp_t = pp.tile([128, 4, 512], DT, name="mp_t")
            wo_t = owp.tile([128, 2, D], DT)

            ones_c = pp.tile([1, 128], DT)
            nc.vector.memset(ones_c.bitcast(DTNP), 1.0)

            ap = ctx_pools.enter_context(tc.tile_pool(name="attn", bufs=5))
            smp = ctx_pools.enter_context(tc.tile_pool(name="small", bufs=3))
            mlp = None if causal else ctx_pools.enter_context(tc.tile_pool(name="mload", bufs=3))
            op = ctx_pools.enter_context(tc.tile_pool(name="outp", bufs=3))
            sp = ctx_pools.enter_context(tc.tile_pool(name="stream", bufs=1))

            q2t = [pp.tile([128, S], DT, tag=f"q2t{i}", name=f"q2t{i}") for i in range(2)]
            k2t = [pp.tile([128, S], DT, tag=f"k2t{i}", name=f"k2t{i}") for i in range(2)]
            # vaug[:, jt, h, 0:64] = v; [.., 64:128] = ones (denominator rows)
            vaug = pp.tile([128, NST, HPC, 128], DT)
            nc.vector.memset(vaug[:, :, :, ds(DH, DH)].bitcast(DTNP), 1.0)
            ctxt = [pp.tile([128, S], DT, tag=f"ctxt{i}", name=f"ctxt{i}") for i in range(2)]

            # ---- per-sb input DMA (double-buffered via bufs=2 tags) ----
            qkv_tiles = {}

            def emit_input_dma(sb):
                qs = sp.tile([128, 8, 512], DT, tag="qs", name="qs", bufs=2)
                ks = sp.tile([128, 8, 512], DT, tag="ks", name="ks", bufs=2)
                vs = sp.tile([128, 8, 512], DT, tag="vs", name="vs", bufs=2)
                nc.sync.dma_start(out=qs, in_=qt[sb].bitcast(DT))
                nc.sync.dma_start(out=ks, in_=kt[sb].bitcast(DT))
                (nc.sync if sb == 1 else nc.gpsimd).dma_start(out=vs, in_=vt[sb].bitcast(DT))
                qkv_tiles[sb] = (qs, ks, vs)

            def proj_unit_qk(sb, hp, which):
                qs, ks, vs = qkv_tiles[sb]
                if which == "q":
                    src, w_t, b_t, dst = qs, wq_t, bq_t, q2t
                else:
                    src, w_t, b_t, dst = ks, wk_t, bk_t, k2t
                ps = mm_ps.tile([128, 512], f32, tag="mm", name="ps")
                for c in range(8):
                    nc.tensor.matmul(
                        ps, w_t[:, c, ts(hp, 128)], src[:, c, :],
                        start=(c == 0), stop=(c == 7),
                    )
                nc.vector.tensor_scalar_add(
                    dst[hp][:, ts(sb, 512)], ps, b_t[:, ds(hp, 1)])

            def proj_unit_v(sb, st4):
                qs, ks, vs = qkv_tiles[sb]
                st = sb * 4 + st4
                psv = mm_ps.tile([128, 256], f32, tag="mm", name="psv")
                for c in range(8):
                    nc.tensor.matmul(
                        psv, vs[:, c, ts(st4, 128)], wv_t[:, c, :],
                        start=(c == 0), stop=False,
                    )
                nc.tensor.matmul(psv, ones_c, bv_t, start=False, stop=True)
                nc.vector.tensor_copy(
                    vaug[:, st, :, 0:DH],
                    psv.rearrange("p (h x) -> p h x", h=HPC),
                )

            def emit_proj(sb):
                for hp in range(2):
                    proj_unit_qk(sb, hp, "q")
                    proj_unit_qk(sb, hp, "k")
                for st4 in range(4):
                    proj_unit_v(sb, st4)
                qkv_tiles.pop(sb)

            fillq = []

            def fill_one():
                if fillq:
                    fillq.pop(0)()

            # ---- attention + deferred output projection ----
            outq = []  # deferred out-proj st units from sb-1

            def emit_out_unit(final=False):
                if not outq:
                    return
                st = outq.pop(0)
                ot = op.tile([128, 1024], DT, tag="ot", name="ot")
                for nb in range(2):
                    po = mm_ps.tile([128, 512], f32, tag="mm", name="po")
                    for k in range(2):
                        nc.tensor.matmul(
                            po, ctxt[k][:, ts(st, 128)],
                            wo_t[:, k, ts(nb, 512)],
                            start=(k == 0), stop=(k == 1),
                        )
                    # scalar is idle after the last exp; share the tail copies
                    if final and nb == 0:
                        nc.scalar.copy(ot[:, ts(nb, 512)], po)
                    else:
                        nc.vector.tensor_copy(ot[:, ts(nb, 512)], po)
                (nc.gpsimd if st % 2 else nc.sync).dma_start(
                    out=out[ts(st, 128), :].bitcast(DT), in_=ot)

            def emit_attn(sb):
                for hp in range(2):
                    jts = list(range(4 * sb + 4)) if causal else list(range(NST))
                    cps = [ctx_ps.tile([128, 512], f32, tag=f"ctx{a}", name=f"cps{a}")
                           for a in range(2)]
                    pending = None  # (jt, at) awaiting attn@v

                    def lo_of(j):
                        # columns si<j0 of tile (j, sb) are fully masked
                        if not causal:
                            return 0
                        return max(0, (j - 4 * sb) * 128)

                    def emit_attnv(pjt, pat, last):
                        lo = lo_of(pjt)
                        for a in range(2):
                            h = 2 * hp + a
                            nc.tensor.matmul(
                                cps[a][:, ds(lo, 512 - lo)],
                                vaug[:, pjt, h, :],
                                pat[:, ds(a * 512 + lo, 512 - lo)],
                                start=(pjt == jts[0]), stop=last,
                            )

                    for jt in jts:
                        emit_out_unit()
                        sc = sc_ps.tile([128, 1024], f32, tag="sc")
                        straddle = causal and jt >= 4 * sb
                        lo = lo_of(jt)
                        mt_t = None
                        if not causal:
                            mt_t = mlp.tile([128, 512], DT, tag="mt")
                            nc.sync.dma_start(
                                out=mt_t,
                                in_=mt[ts(jt, 128), ts(sb, 512)].bitcast(DT))
                        for a in range(2):
                            nc.tensor.matmul(
                                sc[:, ds(a * 512 + lo, 512 - lo)],
                                k2t[hp][ds(a * 64, 64), ts(jt, 128)],
                                q2t[hp][ds(a * 64, 64), ds(sb * 512 + lo, 512 - lo)],
                                start=True, stop=True,
                                tile_position=(a * 64, 0),
                            )
                        at = ap.tile([128, 1024], DT, tag="at")
                        if lo == 0:
                            nc.scalar.activation(at, sc, EXP, scale=0.125)
                        else:
                            # one ACTIVATE over both head-halves via a 3D AP
                            nc.scalar.activation(
                                at.rearrange("p (a x) -> p a x", a=2)[:, :, ds(lo, 512 - lo)],
                                sc.rearrange("p (a x) -> p a x", a=2)[:, :, ds(lo, 512 - lo)],
                                EXP, scale=0.125)
                        if straddle:
                            # only the 128-col diagonal block is partially
                            # masked; columns >= lo+128 are all-ones in mp
                            d = jt - 4 * sb
                            for a in range(2):
                                nc.vector.tensor_mul(
                                    at[:, ds(a * 512 + lo, 128)],
                                    at[:, ds(a * 512 + lo, 128)],
                                    mp_t[:, d, ds(lo, 128)])
                        elif not causal:
                            for a in range(2):
                                nc.vector.tensor_mul(
                                    at[:, ts(a, 512)], at[:, ts(a, 512)], mt_t)
                        if pending is not None:
                            emit_attnv(pending[0], pending[1], False)
                        pending = (jt, at)
                    emit_attnv(pending[0], pending[1], True)
                    for a in range(2):
                        # rows 64:128 of cps all hold the softmax denominator
                        # (reciprocal_approx_fast cannot read PSUM on HW --
                        # stage through SBUF first)
                        dn = smp.tile([DH, 512], f32, tag="dn", name="dn")
                        nc.vector.tensor_copy(dn, cps[a][ds(DH, DH), :])
                        rd = smp.tile([DH, 512], f32, tag="rd", name="rd")
                        nc.vector.reciprocal_approx_fast(rd, dn)
                        nc.vector.tensor_mul(
                            ctxt[hp][ds(a * DH, DH), ts(sb, 512)],
                            cps[a][0:DH, :], rd)
                # queue this sb's out-proj units; drained during next sb
                outq.extend(range(4 * sb, 4 * sb + 4))

            # ---- interleaved emission: proj(sb+1) fills PE gaps of attn(sb) ----
            emit_input_dma(0)
            if causal:
                nc.gpsimd.dma_start(out=mp_t, in_=mp[:, :, :].bitcast(DT))
            nc.gpsimd.dma_start(out=wo_t, in_=wo[:, :, :].bitcast(DT))
            emit_input_dma(1)
            emit_proj(0)
            emit_input_dma(2)
            emit_attn(0)
            emit_proj(1)
            emit_input_dma(3)
            emit_attn(1)
            emit_proj(2)
            emit_attn(2)
            emit_proj(3)
            emit_attn(3)
            while outq:
                emit_out_unit()

            ctx_pools.close()

    nc.finalize()
    _built[key] = nc
    return nc


def _is_causal(masked: np.ndarray) -> bool:
    c = ~np.tril(np.ones((S, S), dtype=bool))
    return all(np.array_equal(masked[b], c) for b in range(masked.shape[0]))


def _swizzle_in(xT: np.ndarray) -> np.ndarray:
    # [D, S] -> [sb, p, c, s] with D = c*128+p, S = sb*512+s
    return np.ascontiguousarray(
        xT.reshape(8, 128, NSB, 512).transpose(2, 1, 0, 3))


def kernel(Q, K, V, masked, WQ_w, WQ_b, WK_w, WK_b, WV_w, WV_b, Wo_w, Wo_b):
    global LAST_RESULT
    from concourse.bass_utils import run_bass_kernel_spmd

    Q = np.asarray(Q, dtype=np.float32)
    K = np.asarray(K, dtype=np.float32)
    V = np.asarray(V, dtype=np.float32)
    masked = np.asarray(masked)
    causal = _is_causal(masked)
    nc = _build(causal, MMDT)
    if MMDT == "f16":
        npdt = np.float16
    elif MMDT == "bf16":
        import ml_dtypes
        npdt = ml_dtypes.bfloat16
    else:
        npdt = np.float32

    qT = [_swizzle_in(Q[b].T.astype(npdt)) for b in range(B)]
    kT = [_swizzle_in(K[b].T.astype(npdt)) for b in range(B)]
    vT = [_swizzle_in(V[b].T.astype(npdt)) for b in range(B)]

    if causal:
        # mp[p, d, f'] multiplicative mask: 0 where (d*128+p) > f
        p = np.arange(128)[:, None, None]
        d = np.arange(4)[None, :, None]
        f = np.arange(512)[None, None, :]
        mp_full = np.ascontiguousarray(
            np.where(d * 128 + p > f, np.float32(0.0), np.float32(1.0)).astype(npdt))
    else:
        mtb = [np.ascontiguousarray(
            np.where(masked[b].T, np.float32(0.0), np.float32(1.0)).astype(npdt))
            for b in range(B)]

    in_maps = []
    for c in range(NCORES):
        b = c // CORES_PER_BATCH
        h0 = (c % CORES_PER_BATCH) * HPC
        sel = slice(h0 * DH, (h0 + HPC) * DH)
        wo_pad = np.asarray(Wo_w).T[sel].reshape(2, 128, D)
        m = {
            "qt": qT[b], "kt": kT[b], "vt": vT[b],
            "wq": np.ascontiguousarray(
                np.asarray(WQ_w)[sel].T.astype(npdt).reshape(8, 128, 256).transpose(1, 0, 2)),
            "wk": np.ascontiguousarray(
                np.asarray(WK_w)[sel].T.astype(npdt).reshape(8, 128, 256).transpose(1, 0, 2)),
            "wv": np.ascontiguousarray(
                np.asarray(WV_w)[sel].T.astype(npdt).reshape(8, 128, 256).transpose(1, 0, 2)),
            "wo": np.ascontiguousarray(wo_pad.astype(npdt).transpose(1, 0, 2)),
            "bq": np.ascontiguousarray(np.asarray(WQ_b)[sel].reshape(2, 128).T.astype(np.float32)),
            "bk": np.ascontiguousarray(np.asarray(WK_b)[sel].reshape(2, 128).T.astype(np.float32)),
            "bv": np.ascontiguousarray(np.asarray(WV_b)[sel].reshape(1, HPC * DH).astype(npdt)),
        }
        if causal:
            m["mp"] = mp_full
        else:
            m["mt"] = mtb[b]
        in_maps.append(m)

    res = run_bass_kernel_spmd(nc, in_maps, core_ids=list(range(NCORES)), trace=TRACE)
    LAST_RESULT = res

    acc = np.zeros((B, S, D), np.float64)
    for c in range(NCORES):
        acc[c // CORES_PER_BATCH] += res.results[c]["out"].astype(np.float64)
    acc += np.asarray(Wo_b, dtype=np.float64)[None, None, :]
    return acc.astype(np.float32)
